# revision 11
# baseline (speedup 1.0000x reference)
"""Bass/Trainium2 kernel for nn_DecoderModel (GPT-2-like, B=4 T=1024 D=1024 H=16 L=12 V=50257).

Sharding: 8 cores; core c handles batch b=c//2, parity p=c%2.
Token rows of each batch are split into 8 tiles of 128; parity p=0 owns global
q-tiles {1,3,5,7}, p=1 owns {0,2,4,6} (balances causal attention flops while
keeping one uniform SPMD instruction stream; per-core additive mask DATA
handles the causal boundary). Residual stream lives transposed in SBUF as
[128, 8, 512] (d-partition, d-tile, token).

v2 changes vs v1:
 - softmax denominator fused into the attention-output matmul via a ones
   column appended to V (stationary [128, 65] -> p_oT row 64 = denominator)
 - all broadcast matmuls in bf16 (v1 used 4-cycle/row f32 matmuls)
 - LM head: weights loaded once (not 4x), no broadcast-bias DMA (lm_b added
   on host), bf16 logits out (host upcast) -> ~615MB/core DMA becomes ~156MB
 - K,V computed before Q; AllGather kicked between, overlapping Q projection
 - optional fp8 (e4m3) DoubleRow attention-output matmul (BASSK_F8ATT=1)
 - per-layer weight DMA batched into column strips
"""
import os
import sys
import math

sys.path.insert(0, "/opt/trn_rl_repo")

import numpy as np
import ml_dtypes

import concourse.bass as bass
import concourse.mybir as mybir
import concourse.tile as tile
from concourse import bacc
from concourse.bass_utils import run_bass_kernel_spmd

BF16 = mybir.dt.bfloat16
F32 = mybir.dt.float32
F8 = mybir.dt.float8e4

B, T, D, H, NL_FULL, V = 4, 1024, 1024, 16, 12, 50257
DH = D // H              # 64
DT = D // 128            # 8 d-tiles
QT = 512 // 128          # 4 q tiles per core
VPAD = 50688             # 99 * 512
NVC = VPAD // 512        # 99 vocab chunks
LN_EPS = 1e-5
INV_SQRT_C = 1.0 / 32.0
NEG = -1.0e9

L = int(os.environ.get("BASSK_L", str(NL_FULL)))
NOAG = os.environ.get("BASSK_NOAG", "0") == "1"   # timing-only: skip collectives
NOLM = os.environ.get("BASSK_NOLM", "0") == "1"   # timing-only: skip LM head
F8ATT = os.environ.get("BASSK_F8ATT", "0") == "1"  # fp8 attention-output matmul

AV = 16.0    # fp8 V scale
ASt = 64.0   # fp8 softmax-weight scale (exp max ~2.3 -> 147 < 240)

KT_ELEMS = D * 512       # K^T block elems in ag buffers
V_ELEMS = 512 * D


def build_nc(num_layers=L):
    nc = bacc.Bacc("TRN2", target_bir_lowering=False, debug=True)
    NLx = num_layers
    att_dt = F8 if F8ATT else BF16

    x0T = nc.declare_dram_parameter("x0T", [128, DT, 512], F32, isOutput=False)
    wq = nc.declare_dram_parameter("wq", [NLx, D, D], BF16, isOutput=False)
    wk = nc.declare_dram_parameter("wk", [NLx, D, D], BF16, isOutput=False)
    wv = nc.declare_dram_parameter("wv", [NLx, D, D], BF16, isOutput=False)
    wo = nc.declare_dram_parameter("wo", [NLx, D, D], BF16, isOutput=False)
    w1 = nc.declare_dram_parameter("w1", [NLx, D, 4 * D], BF16, isOutput=False)
    w2 = nc.declare_dram_parameter("w2", [NLx, 4 * D, D], BF16, isOutput=False)
    ln1s = nc.declare_dram_parameter("ln1s", [NLx, 128, DT], F32, isOutput=False)
    ln1b = nc.declare_dram_parameter("ln1b", [NLx, 128, DT], F32, isOutput=False)
    ln2s = nc.declare_dram_parameter("ln2s", [NLx, 128, DT], F32, isOutput=False)
    ln2b = nc.declare_dram_parameter("ln2b", [NLx, 128, DT], F32, isOutput=False)
    bo_p = nc.declare_dram_parameter("bo_p", [NLx, 128, DT], F32, isOutput=False)
    b1_p = nc.declare_dram_parameter("b1_p", [NLx, 128, 32], F32, isOutput=False)
    b2_p = nc.declare_dram_parameter("b2_p", [NLx, 128, DT], F32, isOutput=False)
    lnfs = nc.declare_dram_parameter("lnfs", [128, DT], F32, isOutput=False)
    lnfb = nc.declare_dram_parameter("lnfb", [128, DT], F32, isOutput=False)
    lmw = nc.declare_dram_parameter("lmw", [D, VPAD], BF16, isOutput=False)
    maskb = nc.declare_dram_parameter("maskb", [2, 128, 128], F32, isOutput=False)
    out = nc.declare_dram_parameter("out", [512, VPAD], BF16, isOutput=True)

    agk_in = [nc.dram_tensor(f"agk_in{i}", [KT_ELEMS], BF16) for i in range(2)]
    agk_out = [nc.dram_tensor(f"agk_out{i}", [2 * KT_ELEMS], BF16) for i in range(2)]
    agv_in = [nc.dram_tensor(f"agv_in{i}", [V_ELEMS], att_dt) for i in range(2)]
    agv_out = [nc.dram_tensor(f"agv_out{i}", [2 * V_ELEMS], att_dt) for i in range(2)]
    groups = [[0, 1], [2, 3], [4, 5], [6, 7]]

    from contextlib import ExitStack
    with tile.TileContext(nc) as tc, ExitStack() as es:
        const = es.enter_context(tc.tile_pool(name="const", bufs=1))
        act32 = es.enter_context(tc.tile_pool(name="act32", bufs=2))
        tbf = es.enter_context(tc.tile_pool(name="tbf", bufs=2))
        nbfp = es.enter_context(tc.tile_pool(name="nbfp", bufs=1))
        proj = es.enter_context(tc.tile_pool(name="proj", bufs=1))
        bigp = es.enter_context(tc.tile_pool(name="bigp", bufs=1))
        wpool = es.enter_context(tc.tile_pool(name="wpool", bufs=4))
        w2pool = es.enter_context(tc.tile_pool(name="w2pool", bufs=2))
        lwpool = es.enter_context(tc.tile_pool(name="lwpool", bufs=2))
        stp = es.enter_context(tc.tile_pool(name="stp", bufs=2))
        ev = es.enter_context(tc.tile_pool(name="ev", bufs=2))
        small = es.enter_context(tc.tile_pool(name="small", bufs=1))
        obfp = es.enter_context(tc.tile_pool(name="obfp", bufs=2))

        ones_bf = const.tile([128, 1], BF16)
        nc.vector.memset(ones_bf[:], 1.0)
        ones_f = const.tile([1, 128], BF16)
        nc.vector.memset(ones_f[:], 1.0)
        ones64 = const.tile([1, 64], BF16)
        nc.vector.memset(ones64[:], (1.0 / AV) if F8ATT else 1.0)
        eps_t = const.tile([1, 1], F32)
        nc.vector.memset(eps_t[:], LN_EPS)
        mask_t = const.tile([128, 2, 128], F32)
        nc.sync.dma_start(mask_t[:], maskb.rearrange("m k q -> k m q"))
        lnf_s_t = const.tile([128, DT], F32)
        nc.sync.dma_start(lnf_s_t[:], lnfs[:])
        lnf_b_t = const.tile([128, DT], F32)
        nc.sync.dma_start(lnf_b_t[:], lnfb[:])

        # persistent gathered K^T / V tiles; V carries a ones column per head
        lnat = const.tile([128, 1], F32)
        nc.vector.memset(lnat[:], math.log(ASt))
        ktf = const.tile([128, 2, DT, 512], BF16, name="ktf")
        vf = const.tile([128, 2, 4, H, 65], att_dt, name="vf")
        nc.vector.memset(vf[:, :, :, :, 64:65], 1.0)

        xT = act32.tile([128, DT, 512], F32, name="xT")
        nc.sync.dma_start(xT[:], x0T[:])

        def layernorm(x_in, s_dram, b_dram):
            """x_in: [128, DT, 512] f32 -> nbf [128, DT, 512] bf16."""
            if s_dram is not None:
                s_t = small.tile([128, DT], F32, name="lns")
                nc.sync.dma_start(s_t[:], s_dram)
                b_t = small.tile([128, DT], F32, name="lnb")
                nc.sync.dma_start(b_t[:], b_dram)
            else:
                s_t, b_t = lnf_s_t, lnf_b_t
            with tc.tile_pool(name="lnp", bufs=2, space="PSUM") as lnp:
                ps1 = lnp.tile([1, 512], F32, name="ps")
                ps2 = lnp.tile([1, 512], F32, name="ps")
                for dt_i in range(DT):
                    xbf = tbf.tile([128, 512], BF16, name="xbf")
                    nc.scalar.copy(xbf[:], x_in[:, dt_i])
                    sq = tbf.tile([128, 512], BF16, name="sq")
                    nc.scalar.square(sq[:], x_in[:, dt_i])
                    nc.tensor.matmul(ps1[:], ones_bf[:], xbf[:],
                                     start=(dt_i == 0), stop=(dt_i == DT - 1))
                    nc.tensor.matmul(ps2[:], ones_bf[:], sq[:],
                                     start=(dt_i == 0), stop=(dt_i == DT - 1))
                mu = small.tile([1, 512], F32, name="mu")
                nc.vector.tensor_scalar_mul(mu[:], ps1[:], 1.0 / D)
                var = small.tile([1, 512], F32, name="var")
                nc.vector.tensor_scalar_mul(var[:], ps2[:], 1.0 / D)
                musq = small.tile([1, 512], F32, name="sd")
                nc.vector.tensor_mul(musq[:], mu[:], mu[:])
                nc.vector.tensor_sub(var[:], var[:], musq[:])
                sd = small.tile([1, 512], F32, name="sd")
                nc.scalar.activation(sd[:], var[:],
                                     mybir.ActivationFunctionType.Sqrt,
                                     bias=eps_t[:])
                rstd = small.tile([1, 512], F32, name="rstd")
                nc.vector.reciprocal(rstd[:], sd[:])
                # broadcasts in bf16 (mu) and bf16 hi+lo (rstd, keeps f32-ish
                # precision on the multiplier that scales the whole stream)
                mu_bf = small.tile([1, 512], BF16, name="recip")
                nc.vector.tensor_scalar_mul(mu_bf[:], mu[:], 1.0)
                rh = small.tile([1, 512], BF16, name="rh")
                nc.vector.tensor_scalar_mul(rh[:], rstd[:], 1.0)
                rl = small.tile([1, 512], BF16, name="rl")
                nc.vector.tensor_sub(rl[:], rstd[:], rh[:])
                mub = lnp.tile([128, 512], F32, name="pb")
                nc.tensor.matmul(mub[:], ones_f[:], mu_bf[:], start=True,
                                 stop=True)
                rstdb = lnp.tile([128, 512], F32, name="pb")
                nc.tensor.matmul(rstdb[:], ones_f[:], rh[:], start=True,
                                 stop=False)
                nc.tensor.matmul(rstdb[:], ones_f[:], rl[:], start=False,
                                 stop=True)
                nbf = nbfp.tile([128, DT, 512], BF16, name="nbf")
                for dt_i in range(DT):
                    t2 = ev.tile([128, 512], F32, name="lntmp")
                    nc.vector.tensor_sub(t2[:], x_in[:, dt_i], mub[:])
                    nc.vector.tensor_mul(t2[:], t2[:], rstdb[:])
                    nc.scalar.activation(nbf[:, dt_i], t2[:],
                                         mybir.ActivationFunctionType.Identity,
                                         bias=b_t[:, dt_i:dt_i + 1],
                                         scale=s_t[:, dt_i:dt_i + 1])
            return nbf

        for l in range(NLx):
            slot = l % 2
            n1_bf = layernorm(xT, ln1s[l], ln1b[l])

            kt_sb = proj.tile([128, DT, 512], BF16, name="kt_sb")
            v_sb = proj.tile([128, 4, D], att_dt, name="v_sb")
            qt_sb = proj.tile([128, DT, 512], BF16, name="qt_sb")
            with tc.tile_pool(name="pqkv", bufs=4, space="PSUM") as pq_pool:
                # ---- K^T projection
                for ft in range(DT):
                    wt = wpool.tile([128, DT, 128], BF16, name="wstrip")
                    nc.sync.dma_start(
                        wt[:], wk[l, :, ft * 128:(ft + 1) * 128].rearrange(
                            "(a p) f -> p a f", p=128))
                    pq = pq_pool.tile([128, 512], F32, name="pq")
                    for dt_i in range(DT):
                        nc.tensor.matmul(
                            pq[:], wt[:, dt_i], n1_bf[:, dt_i],
                            start=(dt_i == 0), stop=(dt_i == DT - 1))
                    nc.vector.tensor_scalar_mul(kt_sb[:, ft], pq[:], 1.0)
                # ---- V projection (token-major, att dtype, AV scale if fp8)
                for half in range(2):
                    pvs = [pq_pool.tile([128, 512], F32, name="pq")
                           for _ in range(4)]
                    for dt_i in range(DT):
                        wt = wpool.tile([128, 512], BF16, name="wvt")
                        nc.sync.dma_start(
                            wt[:], wv[l, dt_i * 128:(dt_i + 1) * 128,
                                      half * 512:(half + 1) * 512])
                        for tt in range(4):
                            nc.tensor.matmul(
                                pvs[tt][:],
                                n1_bf[:, dt_i, tt * 128:(tt + 1) * 128],
                                wt[:], start=(dt_i == 0),
                                stop=(dt_i == DT - 1))
                    for tt in range(4):
                        nc.scalar.activation(
                            v_sb[:, tt, half * 512:(half + 1) * 512], pvs[tt][:],
                            mybir.ActivationFunctionType.Copy,
                            scale=AV if F8ATT else 1.0)

                # ---- kick AllGather of K^T and V within the pair
                kt_dr = agk_in[slot].rearrange("(p a t) -> p a t", p=128, a=DT)
                v_dr = agv_in[slot].rearrange("(p a t) -> p a t", p=128, a=4)
                if NOAG:
                    for blk in range(2):
                        nc.sync.dma_start(
                            agk_out[slot][blk * KT_ELEMS:(blk + 1) * KT_ELEMS]
                            .rearrange("(p a t) -> p a t", p=128, a=DT), kt_sb[:])
                        nc.sync.dma_start(
                            agv_out[slot][blk * V_ELEMS:(blk + 1) * V_ELEMS]
                            .rearrange("(p a t) -> p a t", p=128, a=4), v_sb[:])
                else:
                    nc.sync.dma_start(kt_dr, kt_sb[:])
                    nc.sync.dma_start(v_dr, v_sb[:])
                    nc.gpsimd.collective_compute(
                        "AllGather", mybir.AluOpType.bypass,
                        replica_groups=groups,
                        ins=[agk_in[slot][:]], outs=[agk_out[slot][:]])
                    nc.gpsimd.collective_compute(
                        "AllGather", mybir.AluOpType.bypass,
                        replica_groups=groups,
                        ins=[agv_in[slot][:]], outs=[agv_out[slot][:]])

                # ---- Q^T projection (overlaps the collective)
                for ft in range(DT):
                    wt = wpool.tile([128, DT, 128], BF16, name="wstrip")
                    nc.sync.dma_start(
                        wt[:], wq[l, :, ft * 128:(ft + 1) * 128].rearrange(
                            "(a p) f -> p a f", p=128))
                    pq = pq_pool.tile([128, 512], F32, name="pq")
                    for dt_i in range(DT):
                        nc.tensor.matmul(
                            pq[:], wt[:, dt_i], n1_bf[:, dt_i],
                            start=(dt_i == 0), stop=(dt_i == DT - 1))
                    nc.vector.tensor_scalar_mul(qt_sb[:, ft], pq[:], 1.0)

            # ---- land gathered K^T and V
            for blk in range(2):
                nc.sync.dma_start(
                    ktf[:, blk],
                    agk_out[slot][blk * KT_ELEMS:(blk + 1) * KT_ELEMS]
                    .rearrange("(p a t) -> p a t", p=128, a=DT))
                nc.sync.dma_start(
                    vf[:, blk, :, :, 0:64],
                    agv_out[slot][blk * V_ELEMS:(blk + 1) * V_ELEMS]
                    .rearrange("(p a h e) -> p a h e", p=128, a=4, h=H))

            # ---- attention
            oT_all = proj.tile([128, DT, 512], BF16, name="kt_sb")
            with tc.tile_pool(name="pst", bufs=3, space="PSUM") as pst_pool, \
                 tc.tile_pool(name="pacc", bufs=2, space="PSUM") as pacc, \
                 tc.tile_pool(name="pbc", bufs=2, space="PSUM") as pbc:
                for h in range(H):
                    po = h % 2 * 64
                    rt = h // 2
                    p_oT = pacc.tile([65, 512], F32, name="p_oT")
                    for s in range(4):
                        off = 128 * s
                        st_t = stp.tile([128, 2, 512], att_dt, name="st_t")
                        for blk in range(2):
                            pst = pst_pool.tile([128, 512], F32, name="p_st")
                            nc.tensor.matmul(
                                pst[:, off:],
                                ktf[po:po + 64, blk, rt,
                                    s * 128:(s + 1) * 128],
                                qt_sb[po:po + 64, rt, off:],
                                start=True, stop=True)
                            nc.vector.tensor_add(pst[:, off:off + 128],
                                                 pst[:, off:off + 128],
                                                 mask_t[:, 1 - blk])
                            nc.scalar.activation(
                                st_t[:, blk, off:], pst[:, off:],
                                mybir.ActivationFunctionType.Exp,
                                bias=lnat[:] if F8ATT else 0.0,
                                scale=INV_SQRT_C)
                        if F8ATT:
                            nc.tensor.matmul(
                                p_oT[:, off:], vf[:, :, s, h, :],
                                st_t[:, :, off:],
                                perf_mode=mybir.MatmulPerfMode.DoubleRow,
                                start=(s == 0), stop=(s == 3))
                        else:
                            for blk in range(2):
                                nc.tensor.matmul(
                                    p_oT[:, off:], vf[:, blk, s, h, :],
                                    st_t[:, blk, off:],
                                    start=(s == 0 and blk == 0),
                                    stop=(s == 3 and blk == 1))
                    recip_f = small.tile([1, 512], F32, name="mu")
                    nc.vector.reciprocal(recip_f[:], p_oT[64:65, :])
                    recip = small.tile([1, 512], BF16, name="recip")
                    nc.vector.tensor_scalar_mul(recip[:], recip_f[:], 1.0)
                    p_bc = pbc.tile([64, 512], F32, name="p_bc")
                    nc.tensor.matmul(p_bc[:], ones64[:], recip[:],
                                     start=True, stop=True)
                    bc_sb = ev.tile([64, 512], F32, name="bc_sb")
                    nc.vector.tensor_scalar_mul(bc_sb[:], p_bc[:], 1.0)
                    nc.vector.tensor_mul(oT_all[po:po + 64, rt],
                                         p_oT[0:64, :], bc_sb[:])

            # ---- Wo projection + residual + bo
            bo_t = small.tile([128, DT], F32, name="bo_t")
            nc.sync.dma_start(bo_t[:], bo_p[l])
            x2 = act32.tile([128, DT, 512], F32, name="xT")
            with tc.tile_pool(name="pwo", bufs=3, space="PSUM") as pwo:
                for dt_i in range(DT):
                    wt = wpool.tile([128, DT, 128], BF16, name="wstrip")
                    nc.sync.dma_start(
                        wt[:], wo[l, :, dt_i * 128:(dt_i + 1) * 128].rearrange(
                            "(a p) f -> p a f", p=128))
                    pw = pwo.tile([128, 512], F32, name="pw")
                    for et in range(DT):
                        nc.tensor.matmul(
                            pw[:], wt[:, et], oT_all[:, et],
                            start=(et == 0), stop=(et == DT - 1))
                    nc.vector.scalar_tensor_tensor(
                        x2[:, dt_i], pw[:], bo_t[:, dt_i:dt_i + 1],
                        n1_bf[:, dt_i],
                        mybir.AluOpType.add, mybir.AluOpType.add)

            n2_bf = layernorm(x2, ln2s[l], ln2b[l])

            # ---- MLP
            b1_t = small.tile([128, 32], F32, name="b1_t")
            nc.sync.dma_start(b1_t[:], b1_p[l])
            hT = bigp.tile([128, 32, 512], BF16, name="hT")
            with tc.tile_pool(name="pmlp", bufs=4, space="PSUM") as pmlp:
                for ht in range(32):
                    wt = wpool.tile([128, DT, 128], BF16, name="wstrip")
                    nc.sync.dma_start(
                        wt[:], w1[l, :, ht * 128:(ht + 1) * 128].rearrange(
                            "(a p) f -> p a f", p=128))
                    ph = pmlp.tile([128, 512], F32, name="ph")
                    for dt_i in range(DT):
                        nc.tensor.matmul(ph[:], wt[:, dt_i], n2_bf[:, dt_i],
                                         start=(dt_i == 0),
                                         stop=(dt_i == DT - 1))
                    # relu(ph + b1) on the vector engine
                    nc.vector.tensor_scalar(hT[:, ht], ph[:],
                                            b1_t[:, ht:ht + 1], 0.0,
                                            mybir.AluOpType.add,
                                            mybir.AluOpType.max)
                b2_t = small.tile([128, DT], F32, name="b2_t")
                nc.sync.dma_start(b2_t[:], b2_p[l])
                x3 = act32.tile([128, DT, 512], F32, name="xT")
                for dt_i in range(DT):
                    wt = w2pool.tile([128, 32, 128], BF16, name="w2strip")
                    nc.sync.dma_start(
                        wt[:], w2[l, :, dt_i * 128:(dt_i + 1) * 128].rearrange(
                            "(a p) f -> p a f", p=128))
                    py = pmlp.tile([128, 512], F32, name="ph")
                    for ht in range(32):
                        nc.tensor.matmul(py[:], wt[:, ht], hT[:, ht],
                                         start=(ht == 0), stop=(ht == 31))
                    nc.vector.scalar_tensor_tensor(
                        x3[:, dt_i], py[:], b2_t[:, dt_i:dt_i + 1],
                        n2_bf[:, dt_i],
                        mybir.AluOpType.add, mybir.AluOpType.add)
            xT = x3

        # ---- final LN + LM head (lm_b added on host)
        nf_bf = layernorm(xT, None, None)
        if not NOLM:
            with tc.tile_pool(name="plm", bufs=8, space="PSUM") as plm:
                for vg in range(0, NVC, 2):
                    vis = [vi for vi in (vg, vg + 1) if vi < NVC]
                    lws = []
                    for vi in vis:
                        lw = lwpool.tile([128, DT, 512], BF16, name="lw")
                        nc.sync.dma_start(
                            lw[:], lmw[:, vi * 512:(vi + 1) * 512].rearrange(
                                "(a p) f -> p a f", p=128))
                        lws.append(lw)
                    pls = {(vi, q): plm.tile([128, 512], F32, name="p_lm")
                           for vi in vis for q in range(QT)}
                    for dt_i in range(DT):
                        for j, vi in enumerate(vis):
                            for q in range(QT):
                                nc.tensor.matmul(
                                    pls[(vi, q)][:],
                                    nf_bf[:, dt_i, q * 128:(q + 1) * 128],
                                    lws[j][:, dt_i], start=(dt_i == 0),
                                    stop=(dt_i == DT - 1))
                    for vi in vis:
                        for q in range(QT):
                            ot = obfp.tile([128, 512], BF16, name="ot")
                            nc.vector.tensor_scalar_mul(ot[:], pls[(vi, q)][:],
                                                        1.0)
                            nc.sync.dma_start(
                                out[q * 128:(q + 1) * 128,
                                    vi * 512:(vi + 1) * 512], ot[:])

    nc.compile()
    return nc


def host_prep(inputs, num_layers=L):
    """Build per-core in_maps + reassembly metadata from full inputs."""
    f32 = np.float32
    bf = ml_dtypes.bfloat16
    idx = np.asarray(inputs["idx"])
    tok_emb = np.asarray(inputs["tok_emb"], f32)
    pos_emb = np.asarray(inputs["pos_emb"], f32)

    def perD(a):  # [L?, D] -> [L?, 128, DT]
        a = np.asarray(a, f32)
        if a.ndim == 1:
            return np.ascontiguousarray(a.reshape(DT, 128).T)
        return np.ascontiguousarray(a.reshape(a.shape[0], -1, 128).transpose(0, 2, 1))

    wq = np.ascontiguousarray(np.asarray(inputs["Wq"], f32)[:num_layers]).astype(bf)
    wk = np.ascontiguousarray(np.asarray(inputs["Wk"], f32)[:num_layers]).astype(bf)
    wv = np.ascontiguousarray(np.asarray(inputs["Wv"], f32)[:num_layers]).astype(bf)
    wo = np.ascontiguousarray(np.asarray(inputs["Wo"], f32)[:num_layers]).astype(bf)
    w1 = np.ascontiguousarray(np.asarray(inputs["W1"], f32)[:num_layers]).astype(bf)
    w2 = np.ascontiguousarray(np.asarray(inputs["W2"], f32)[:num_layers]).astype(bf)
    ln1s = perD(inputs["ln1_s"])[:num_layers]
    ln1b = perD(inputs["ln1_b"])[:num_layers]
    ln2s = perD(inputs["ln2_s"])[:num_layers]
    ln2b = perD(inputs["ln2_b"])[:num_layers]
    bo_p = perD(inputs["bo"])[:num_layers]
    b1_p = perD(inputs["b1"])[:num_layers]
    b2_p = perD(inputs["b2"])[:num_layers]
    lnfs = perD(inputs["lnf_s"])
    lnfb = perD(inputs["lnf_b"])
    lmw = np.zeros((D, VPAD), f32)
    lmw[:, :V] = np.asarray(inputs["lm_W"], f32)
    lmw = lmw.astype(bf)

    # additive causal masks, [k, q] on the diagonal 128-block
    kk = np.arange(128)[:, None]
    qq = np.arange(128)[None, :]
    triadd = np.where(kk <= qq, 0.0, NEG).astype(f32)
    zeros = np.zeros((128, 128), f32)
    fullneg = np.full((128, 128), NEG, f32)

    in_maps = []
    tiles_by_parity = []
    for c in range(8):
        b, p = c // 2, c % 2
        g_tiles = [2 * j + 1 - p for j in range(QT)]
        tiles_by_parity.append(g_tiles)
        rows = np.concatenate([np.arange(g * 128, (g + 1) * 128) for g in g_tiles])
        x0 = tok_emb[idx[b, rows]] + pos_emb[rows]          # [512, D]
        x0T = np.ascontiguousarray(
            x0.T.reshape(DT, 128, 512).transpose(1, 0, 2)).astype(f32)
        # maskb[j] is the additive mask for k-tiles with global parity j
        if p == 0:
            masks = np.stack([zeros, triadd])
        else:
            masks = np.stack([triadd, fullneg])
        in_maps.append(dict(
            x0T=x0T, wq=wq, wk=wk, wv=wv, wo=wo, w1=w1, w2=w2,
            ln1s=ln1s, ln1b=ln1b, ln2s=ln2s, ln2b=ln2b,
            bo_p=bo_p, b1_p=b1_p, b2_p=b2_p, lnfs=lnfs, lnfb=lnfb,
            lmw=lmw, maskb=masks,
        ))
    return in_maps, tiles_by_parity


def assemble(results, tiles_by_parity, lm_b):
    out = np.empty((B, T, V), np.float32)
    lmb = np.asarray(lm_b, np.float32)
    for c in range(8):
        b = c // 2
        co = results[c]["out"].astype(np.float32)
        for j, g in enumerate(tiles_by_parity[c]):
            out[b, g * 128:(g + 1) * 128] = co[j * 128:(j + 1) * 128, :V] + lmb
    return out


_CACHE = {}


def run(inputs, num_layers=L, trace=False):
    in_maps, tiles = host_prep(inputs, num_layers)
    key = num_layers
    if key not in _CACHE:
        _CACHE[key] = build_nc(num_layers)
    nc = _CACHE[key]
    res = run_bass_kernel_spmd(nc, in_maps, core_ids=list(range(8)), trace=trace)
    return assemble(res.results, tiles, inputs["lm_b"]), res


def kernel(**inputs):
    out, _ = run(inputs, L)
    return out
